# revision 12
# baseline (speedup 1.0000x reference)
"""Trainium2 Bass kernel for nn_CrossAttentionLayer (sparse windowed cross-attention).

Math (per batch b):
  q = hidden @ Wq.T + bq ; k = cross @ Wk.T + bk ; v = cross @ Wv.T + bv
  scores = (q k^T) * HD^-0.5 per head, masked to |i-j| <= 64
  attn = softmax(scores) @ v ; attn = attn @ Wo.T + bo
  gate = sigmoid(hidden @ Wg.T + bg) ; attn = gate * attn
  out = LN(0.5*hidden + 0.5*attn) * gamma + beta   (LN is scale-invariant ->
        computed as LN(hidden + gate*attn))

Sharding: data-parallel over batch. 16 sequences -> 8 cores x 2 sequences.
All matmuls in bf16 with f32 PSUM accumulation; residual + LayerNorm in f32.

Device-side layouts per core (T = 1024 tokens = 2 seqs x 512):
  h32  [T, D]  f32   token-major hidden (residual path)
  hT   [D, T]  bf16  hidden transposed (host-pretransposed, matmul lhs/rhs)
  cT   [D, T]  bf16  cross transposed
  w*T  [D, D]  bf16  transposed weights (in-dim on partitions)
  qT/kT feature-major [D, T]; v/gate token-major [T, D]
"""

import sys

import numpy as np

sys.path.insert(0, "/opt/trn_rl_repo")

import concourse.bass as bass
import concourse.mybir as mybir
import concourse.tile as tile
from concourse import bacc
from concourse.bass_utils import run_bass_kernel_spmd

import ml_dtypes

F32 = mybir.dt.float32
BF16 = mybir.dt.bfloat16
AF = mybir.ActivationFunctionType
ALU = mybir.AluOpType

H = 16
D = 1024
HD = 64
S = 512
B = 16
NCORES = 8
SEQ_PER_CORE = B // NCORES      # 2
T = SEQ_PER_CORE * S            # 1024 tokens per core
SCALE = HD ** -0.5
W2 = 64                         # half window
NEG = -30000.0                  # post-scale additive mask value; exp() -> 0.0
P = 128
NT = T // P                     # 8 token tiles per core
ND = D // P                     # 8 feature chunks
QB = S // P                     # 4 query blocks per sequence
KW = 384                        # aligned key window (3 key tiles)
LN_EPS = 1e-5

_CACHE = {}


def _build_masks():
    """4 additive mask tiles [P, KW] f32 (one per query block of a sequence).

    qb 0,1 use key tiles 0..2 of the sequence; qb 2,3 use key tiles 1..3.
    mask[qb][i, j] = 0 if |(qb*128+i) - (base_kt*128+j)| <= 64 else NEG
    """
    masks = np.full((QB, P, KW), NEG, dtype=np.float32)
    for qb in range(QB):
        base_kt = 0 if qb < 2 else 1
        i = qb * P + np.arange(P)[:, None]
        j = base_kt * P + np.arange(KW)[None, :]
        masks[qb][np.abs(i - j) <= W2] = 0.0
    return masks


def _build_program():
    nc = bacc.Bacc("TRN2", target_bir_lowering=False, debug=False)

    h32_d = nc.declare_dram_parameter("h32", [T, D], F32, isOutput=False)
    hT_d = nc.declare_dram_parameter("hT", [D, T], BF16, isOutput=False)
    cT_d = nc.declare_dram_parameter("cT", [D, T], BF16, isOutput=False)
    wqT_d = nc.declare_dram_parameter("wqT", [D, D], BF16, isOutput=False)
    wkT_d = nc.declare_dram_parameter("wkT", [D, D], BF16, isOutput=False)
    wvT_d = nc.declare_dram_parameter("wvT", [D, D], BF16, isOutput=False)
    wgT_d = nc.declare_dram_parameter("wgT", [D, D], BF16, isOutput=False)
    woT_d = nc.declare_dram_parameter("woT", [D, D], BF16, isOutput=False)
    bqs_d = nc.declare_dram_parameter("bqs", [P, ND], F32, isOutput=False)
    bks_d = nc.declare_dram_parameter("bks", [P, ND], F32, isOutput=False)
    bv_d = nc.declare_dram_parameter("bv", [D], F32, isOutput=False)
    bg_d = nc.declare_dram_parameter("bg", [D], F32, isOutput=False)
    bo_d = nc.declare_dram_parameter("bo", [D], F32, isOutput=False)
    gamma_d = nc.declare_dram_parameter("gamma", [D], F32, isOutput=False)
    beta_d = nc.declare_dram_parameter("beta", [D], F32, isOutput=False)
    masks_d = nc.declare_dram_parameter("masks", [QB, P, KW], F32, isOutput=False)
    ident_d = nc.declare_dram_parameter("ident", [P, P], BF16, isOutput=False)
    out_d = nc.declare_dram_parameter("out", [T, D], F32, isOutput=True)

    def bcast(vec_d):
        # [D] dram vector -> [P, D] AP with 0-stride partition dim (DMA broadcast)
        a = vec_d[:]
        return bass.AP(tensor=a.tensor, offset=a.offset, ap=[[0, P], *a.ap])

    with tile.TileContext(nc) as tc:
        from contextlib import ExitStack

        with ExitStack() as ctx:
            consts = ctx.enter_context(tc.tile_pool(name="consts", bufs=1))
            persist = ctx.enter_context(tc.tile_pool(name="persist", bufs=1))
            psum = ctx.enter_context(tc.tile_pool(name="psum", bufs=1, space="PSUM"))
            work = ctx.enter_context(tc.tile_pool(name="work", bufs=2))

            # ---- constants ----
            masks_sb = []
            for qb in range(QB):
                m = consts.tile([P, KW], F32, tag=f"mask{qb}", name=f"mask{qb}")
                nc.sync.dma_start(out=m, in_=masks_d[qb])
                masks_sb.append(m)
            ident = consts.tile([P, P], BF16, tag="ident", name="ident")
            nc.sync.dma_start(out=ident, in_=ident_d[:])
            bqs = consts.tile([P, ND], F32, tag="bqs", name="bqs")
            nc.sync.dma_start(out=bqs, in_=bqs_d[:])
            bks = consts.tile([P, ND], F32, tag="bks", name="bks")
            nc.sync.dma_start(out=bks, in_=bks_d[:])
            # attention-path biases can be bf16 (error budget is dominated by
            # the f32 residual); gamma/beta stay f32 (they scale the output)
            bv_bc = consts.tile([P, D], F32, tag="bv_bc", name="bv_bc")
            nc.sync.dma_start(out=bv_bc, in_=bcast(bv_d))
            bg_bc = consts.tile([P, D], F32, tag="bg_bc", name="bg_bc")
            nc.sync.dma_start(out=bg_bc, in_=bcast(bg_d))
            bo_bc = consts.tile([P, D], F32, tag="bo_bc", name="bo_bc")
            nc.sync.dma_start(out=bo_bc, in_=bcast(bo_d))
            gamma_bc = consts.tile([P, D], F32, tag="gamma_bc", name="gamma_bc")
            nc.sync.dma_start(out=gamma_bc, in_=bcast(gamma_d))
            beta_bc = consts.tile([P, D], F32, tag="beta_bc", name="beta_bc")
            nc.sync.dma_start(out=beta_bc, in_=bcast(beta_d))
            eps_sb = consts.tile([P, 1], F32, tag="eps", name="eps")
            nc.vector.memset(eps_sb, LN_EPS)

            # ---- persistent activation tensors ----
            kT = [persist.tile([P, T], BF16, tag=f"kT{i}", name=f"kT{i}") for i in range(ND)]
            v = [persist.tile([P, D], BF16, tag=f"v{i}", name=f"v{i}") for i in range(NT)]
            qT = [persist.tile([P, T], BF16, tag=f"qT{i}", name=f"qT{i}") for i in range(ND)]
            gate = [persist.tile([P, D], BF16, tag=f"g{i}", name=f"g{i}") for i in range(NT)]

            # ================= phase 1: K, V from cross =================
            with tc.tile_pool(name="ph1", bufs=1) as ph1:
                cT_sb = []
                wkT_sb = []
                wvT_sb = []
                for dk in range(ND):
                    t_ = ph1.tile([P, T], BF16, tag=f"cT{dk}", name=f"cT{dk}")
                    nc.sync.dma_start(out=t_, in_=cT_d[dk * P:(dk + 1) * P, :])
                    cT_sb.append(t_)
                    t_ = ph1.tile([P, D], BF16, tag=f"wkT{dk}", name=f"wkT{dk}")
                    nc.sync.dma_start(out=t_, in_=wkT_d[dk * P:(dk + 1) * P, :])
                    wkT_sb.append(t_)
                    t_ = ph1.tile([P, D], BF16, tag=f"wvT{dk}", name=f"wvT{dk}")
                    nc.sync.dma_start(out=t_, in_=wvT_d[dk * P:(dk + 1) * P, :])
                    wvT_sb.append(t_)

                # kT[oc][:, th*512:...] = sum_dk wkT[dk][:,oc-cols].T @ cT[dk][:, th-cols]
                for oc in range(ND):
                    for th in range(2):
                        ps = psum.tile([P, 512], F32, tag="proj", bufs=2, name="ps_k")
                        for dk in range(ND):
                            nc.tensor.matmul(
                                ps,
                                lhsT=wkT_sb[dk][:, oc * P:(oc + 1) * P],
                                rhs=cT_sb[dk][:, th * 512:(th + 1) * 512],
                                start=(dk == 0), stop=(dk == ND - 1),
                            )
                        nc.scalar.activation(
                            out=kT[oc][:, th * 512:(th + 1) * 512], in_=ps,
                            func=AF.Identity, bias=bks[:, oc:oc + 1], scale=1.0,
                        )

                # v[tt][:, oh*512:...] = sum_dk cT[dk][:, tt-cols].T @ wvT[dk][:, oh-cols]
                for tt in range(NT):
                    for oh in range(2):
                        ps = psum.tile([P, 512], F32, tag="proj", bufs=2, name="ps_v")
                        for dk in range(ND):
                            nc.tensor.matmul(
                                ps,
                                lhsT=cT_sb[dk][:, tt * P:(tt + 1) * P],
                                rhs=wvT_sb[dk][:, oh * 512:(oh + 1) * 512],
                                start=(dk == 0), stop=(dk == ND - 1),
                            )
                        nc.vector.tensor_add(
                            out=v[tt][:, oh * 512:(oh + 1) * 512],
                            in0=ps, in1=bv_bc[:, oh * 512:(oh + 1) * 512],
                        )

            # ================= phase 2: Q, gate from hidden =================
            with tc.tile_pool(name="ph2", bufs=1) as ph2:
                hT_sb = []
                wqT_sb = []
                wgT_sb = []
                for dk in range(ND):
                    t_ = ph2.tile([P, T], BF16, tag=f"hT{dk}", name=f"hT{dk}")
                    nc.sync.dma_start(out=t_, in_=hT_d[dk * P:(dk + 1) * P, :])
                    hT_sb.append(t_)
                    t_ = ph2.tile([P, D], BF16, tag=f"wqT{dk}", name=f"wqT{dk}")
                    nc.sync.dma_start(out=t_, in_=wqT_d[dk * P:(dk + 1) * P, :])
                    wqT_sb.append(t_)
                    t_ = ph2.tile([P, D], BF16, tag=f"wgT{dk}", name=f"wgT{dk}")
                    nc.sync.dma_start(out=t_, in_=wgT_d[dk * P:(dk + 1) * P, :])
                    wgT_sb.append(t_)
                for oc in range(ND):
                    for th in range(2):
                        ps = psum.tile([P, 512], F32, tag="proj", bufs=2, name="ps_q")
                        for dk in range(ND):
                            nc.tensor.matmul(
                                ps,
                                lhsT=wqT_sb[dk][:, oc * P:(oc + 1) * P],
                                rhs=hT_sb[dk][:, th * 512:(th + 1) * 512],
                                start=(dk == 0), stop=(dk == ND - 1),
                            )
                        nc.scalar.activation(
                            out=qT[oc][:, th * 512:(th + 1) * 512], in_=ps,
                            func=AF.Identity, bias=bqs[:, oc:oc + 1], scale=1.0,
                        )

                for tt in range(NT):
                    for oh in range(2):
                        ps = psum.tile([P, 512], F32, tag="proj", bufs=2, name="ps_g")
                        for dk in range(ND):
                            nc.tensor.matmul(
                                ps,
                                lhsT=hT_sb[dk][:, tt * P:(tt + 1) * P],
                                rhs=wgT_sb[dk][:, oh * 512:(oh + 1) * 512],
                                start=(dk == 0), stop=(dk == ND - 1),
                            )
                        gtmp = work.tile([P, 512], F32, tag="gtmp", name="gtmp")
                        nc.vector.tensor_add(
                            out=gtmp, in0=ps, in1=bg_bc[:, oh * 512:(oh + 1) * 512],
                        )
                        nc.scalar.activation(
                            out=gate[tt][:, oh * 512:(oh + 1) * 512], in_=gtmp,
                            func=AF.Sigmoid,
                        )

            # ================= phase 3: attention + out proj + epilogue =================
            with tc.tile_pool(name="ph3", bufs=1) as ph3:
                woT_sb = []
                for dk in range(ND):
                    t_ = ph3.tile([P, D], BF16, tag=f"woT{dk}", name=f"woT{dk}")
                    nc.sync.dma_start(out=t_, in_=woT_d[dk * P:(dk + 1) * P, :])
                    woT_sb.append(t_)

                for tt in range(NT):
                    s = tt // QB
                    qb = tt % QB
                    base_kt = (0 if qb < 2 else 1) + s * QB  # global key tile base

                    attnT = work.tile([P, ND, P], BF16, tag="attnT", name=f"attnT{tt}")
                    for c in range(ND):  # head pair -> feature chunk c
                        ps_aT = psum.tile([P, P], F32, tag="aT", bufs=1, name="ps_aT")
                        for u in range(2):
                            h = 2 * c + u
                            oc = h // 2
                            row0 = (h % 2) * HD
                            # scores [128 q, 384 k]
                            ps_s = psum.tile([P, KW], F32, tag="ps", bufs=2, name="ps_s")
                            nc.tensor.matmul(
                                ps_s,
                                lhsT=qT[oc][row0:row0 + HD, tt * P:(tt + 1) * P],
                                rhs=kT[oc][row0:row0 + HD,
                                           base_kt * P:base_kt * P + KW],
                                start=True, stop=True,
                            )
                            # scaled scores + additive mask (f32)
                            sc32 = work.tile([P, KW], F32, tag="sc32", name="sc32")
                            nc.vector.scalar_tensor_tensor(
                                out=sc32, in0=ps_s, scalar=SCALE,
                                in1=masks_sb[qb], op0=ALU.mult, op1=ALU.add,
                            )
                            # exp + row-sum
                            probs = work.tile([P, KW], BF16, tag="probs", name="probs")
                            den = work.tile([P, 1], F32, tag="den", name="den")
                            nc.scalar.activation(
                                out=probs, in_=sc32, func=AF.Exp, accum_out=den,
                            )
                            rden = work.tile([P, 1], F32, tag="rden", name="rden")
                            nc.vector.reciprocal(out=rden, in_=den)
                            probs_n = work.tile([P, KW], BF16, tag="probs_n",
                                                name="probs_n")
                            nc.vector.tensor_scalar_mul(
                                out=probs_n, in0=probs, scalar1=rden,
                            )
                            # transpose probs (3 chunks) via identity matmul
                            ps_pT = psum.tile([P, KW], F32, tag="pT", bufs=2,
                                              name="ps_pT")
                            for j in range(3):
                                nc.tensor.matmul(
                                    ps_pT[:, j * P:(j + 1) * P],
                                    lhsT=probs_n[:, j * P:(j + 1) * P],
                                    rhs=ident,
                                    start=True, stop=True,
                                )
                            probsT = work.tile([P, KW], BF16, tag="probsT",
                                               name="probsT")
                            nc.scalar.activation(out=probsT, in_=ps_pT, func=AF.Copy)
                            # attnT[hd, q] for this head
                            for j in range(3):
                                kt = base_kt + j
                                nc.tensor.matmul(
                                    ps_aT[u * HD:(u + 1) * HD, :],
                                    lhsT=v[kt][:, h * HD:(h + 1) * HD],
                                    rhs=probsT[:, j * P:(j + 1) * P],
                                    start=(j == 0), stop=(j == 2),
                                )
                        nc.scalar.activation(out=attnT[:, c, :], in_=ps_aT,
                                             func=AF.Copy)

                    # out projection + epilogue for this token tile
                    h32t = work.tile([P, D], F32, tag="h32t", name="h32t")
                    nc.sync.dma_start(out=h32t, in_=h32_d[tt * P:(tt + 1) * P, :])
                    ta = work.tile([P, D], F32, tag="ta", name="ta")
                    for oh in range(2):
                        ps_o = psum.tile([P, 512], F32, tag="o", bufs=1, name="ps_o")
                        for c in range(ND):
                            nc.tensor.matmul(
                                ps_o,
                                lhsT=attnT[:, c, :],
                                rhs=woT_sb[c][:, oh * 512:(oh + 1) * 512],
                                start=(c == 0), stop=(c == ND - 1),
                            )
                        nc.vector.tensor_add(
                            out=ta[:, oh * 512:(oh + 1) * 512], in0=ps_o,
                            in1=bo_bc[:, oh * 512:(oh + 1) * 512],
                        )
                    # gated residual: pre = hidden + gate*attn  (LN scale-invariant)
                    tb = work.tile([P, D], F32, tag="tb", name="tb")
                    nc.vector.tensor_mul(out=ta, in0=ta, in1=gate[tt])
                    nc.vector.tensor_add(out=tb, in0=ta, in1=h32t)
                    # LayerNorm
                    stats = work.tile([P, 2, 6], F32, tag="stats", name="stats")
                    for half in range(2):
                        nc.vector.bn_stats(out=stats[:, half, :],
                                           in_=tb[:, half * 512:(half + 1) * 512])
                    mv = work.tile([P, 2], F32, tag="mv", name="mv")
                    nc.vector.bn_aggr(out=mv, in_=stats)
                    std = work.tile([P, 1], F32, tag="std", name="std")
                    nc.scalar.activation(out=std, in_=mv[:, 1:2], func=AF.Sqrt,
                                         bias=eps_sb, scale=1.0)
                    rstd = work.tile([P, 1], F32, tag="rstd", name="rstd")
                    nc.vector.reciprocal(out=rstd, in_=std)
                    # (tb - mean) * gamma -> ta, then * rstd in place, + beta -> tb
                    nc.vector.scalar_tensor_tensor(
                        out=ta, in0=tb, scalar=mv[:, 0:1], in1=gamma_bc,
                        op0=ALU.subtract, op1=ALU.mult,
                    )
                    nc.vector.tensor_scalar_mul(out=ta, in0=ta, scalar1=rstd)
                    nc.vector.tensor_add(out=tb, in0=ta, in1=beta_bc)
                    nc.sync.dma_start(out=out_d[tt * P:(tt + 1) * P, :], in_=tb)

    nc.compile()
    return nc


def _prep_host(inputs):
    bf = ml_dtypes.bfloat16
    hidden = np.ascontiguousarray(inputs["hidden_states"], dtype=np.float32)
    cross = np.ascontiguousarray(inputs["cross_states"], dtype=np.float32)
    shared = {
        "wqT": np.ascontiguousarray(inputs["Wq"].T).astype(bf),
        "wkT": np.ascontiguousarray(inputs["Wk"].T).astype(bf),
        "wvT": np.ascontiguousarray(inputs["Wv"].T).astype(bf),
        "wgT": np.ascontiguousarray(inputs["Wg"].T).astype(bf),
        "woT": np.ascontiguousarray(inputs["Wo"].T).astype(bf),
        "bqs": np.ascontiguousarray(
            inputs["bq"].astype(np.float32).reshape(ND, P).T),
        "bks": np.ascontiguousarray(
            inputs["bk"].astype(np.float32).reshape(ND, P).T),
        "bv": inputs["bv"].astype(np.float32),
        "bg": inputs["bg"].astype(np.float32),
        "bo": inputs["bo"].astype(np.float32),
        "gamma": inputs["gamma"].astype(np.float32),
        "beta": inputs["beta"].astype(np.float32),
        "masks": _build_masks(),
        "ident": np.eye(P, dtype=bf),
    }
    in_maps = []
    for core in range(NCORES):
        hs = hidden[core * SEQ_PER_CORE:(core + 1) * SEQ_PER_CORE].reshape(T, D)
        cs = cross[core * SEQ_PER_CORE:(core + 1) * SEQ_PER_CORE].reshape(T, D)
        m = dict(shared)
        m["h32"] = np.ascontiguousarray(hs)
        m["hT"] = np.ascontiguousarray(hs.T).astype(bf)
        m["cT"] = np.ascontiguousarray(cs.T).astype(bf)
        in_maps.append(m)
    return in_maps


def _run(inputs, trace=False):
    if "nc" not in _CACHE:
        _CACHE["nc"] = _build_program()
    nc = _CACHE["nc"]
    in_maps = _prep_host(inputs)
    res = run_bass_kernel_spmd(nc, in_maps, list(range(NCORES)), trace=trace)
    out = np.empty((B, S, D), dtype=np.float32)
    for core in range(NCORES):
        out[core * SEQ_PER_CORE:(core + 1) * SEQ_PER_CORE] = (
            np.asarray(res.results[core]["out"], dtype=np.float32).reshape(
                SEQ_PER_CORE, S, D))
    return out, res


def kernel(**inputs):
    out, _ = _run(inputs, trace=False)
    return out


def bench(inputs, iters=20):
    """Amortized device-time benchmark: device-resident inputs, N back-to-back
    dispatches, report per-iteration wall time."""
    import time

    import jax
    from jax.sharding import Mesh, NamedSharding, PartitionSpec
    from jax.experimental.shard_map import shard_map
    from concourse import bass2jax, mybir as _mybir

    if "nc" not in _CACHE:
        _CACHE["nc"] = _build_program()
    nc = _CACHE["nc"]
    in_maps = _prep_host(inputs)
    bass2jax.install_neuronx_cc_hook()

    partition_name = (nc.partition_id_tensor.name if nc.partition_id_tensor
                      else None)
    in_names, out_names, out_avals, zero_outs = [], [], [], []
    for alloc in nc.m.functions[0].allocations:
        if not isinstance(alloc, _mybir.MemoryLocationSet):
            continue
        name = alloc.memorylocations[0].name
        if alloc.kind == "ExternalInput":
            if name != partition_name:
                in_names.append(name)
        elif alloc.kind == "ExternalOutput":
            out_names.append(name)
            shape = tuple(alloc.tensor_shape)
            dtype = _mybir.dt.np(alloc.dtype)
            out_avals.append(jax.core.ShapedArray(shape, dtype))
            zero_outs.append(np.zeros(shape, dtype))
    n_params = len(in_names)
    all_in_names = in_names + out_names
    if partition_name is not None:
        all_in_names.append(partition_name)

    def _body(*args):
        operands = list(args)
        if partition_name is not None:
            operands.append(bass2jax.partition_id_tensor())
        outs = bass2jax._bass_exec_p.bind(
            *operands,
            out_avals=tuple(out_avals),
            in_names=tuple(all_in_names),
            out_names=tuple(out_names),
            lowering_input_output_aliases=(),
            sim_require_finite=True,
            sim_require_nnan=True,
            nc=nc,
        )
        return tuple(outs)

    devices = jax.devices()[:NCORES]
    mesh = Mesh(np.asarray(devices), ("core",))
    spec = PartitionSpec("core")
    n_outs = len(out_names)
    sharded = jax.jit(
        shard_map(_body, mesh=mesh, in_specs=(spec,) * (n_params + n_outs),
                  out_specs=(spec,) * n_outs, check_rep=False),
        keep_unused=True,
    )
    concat_in = [
        np.concatenate([np.asarray(in_maps[c][name]) for c in range(NCORES)], axis=0)
        for name in in_names
    ]
    concat_zero = [np.zeros((NCORES * z.shape[0], *z.shape[1:]), z.dtype)
                   for z in zero_outs]
    sh = NamedSharding(mesh, spec)
    dev_in = [jax.device_put(a, sh) for a in concat_in]
    dev_zero = [jax.device_put(a, sh) for a in concat_zero]

    # warmup (compile)
    out = sharded(*dev_in, *dev_zero)
    jax.block_until_ready(out)
    t0 = time.perf_counter()
    for _ in range(iters):
        out = sharded(*dev_in, *dev_zero)
    jax.block_until_ready(out)
    t1 = time.perf_counter()
    per_iter_ns = (t1 - t0) / iters * 1e9
    return per_iter_ns, out
